# revision 8
# baseline (speedup 1.0000x reference)
"""GRU cell on 8 Trainium2 NeuronCores.

Reference computation (B=65536, D=256):
    z = sigmoid(x@Wz + h@Uz + bz)
    r = sigmoid(x@Wr + h@Ur + br)
    h_hat = tanh(x@Wh + (r*h)@Uh + bh)
    h_t = z*h + (1-z)*h_hat  ; returns (h_t, h_t)

Strategy: data-parallel over the batch dim (8 shards of 8192 rows), all
fp16 on chip (rel_l2 ~1.1e-3 vs the f32 reference; gate is 2e-2):
  * host packs each shard as [128 partitions, 4 blocks, 8192] fp16 where
    the blocks are (x k0, x k1, h k0, h k1) - the contraction dim of all
    six GEMMs is the SBUF partition dim and one DMA fetches all four
    operand tiles of a column range (DMA triggers are ~645ns each on
    SyncE, so trigger count is latency that delays the pipeline head),
  * weights packed into one [256, 1536] fp16 matrix, r-gate slice first,
    DMA-ordered so the first matmul can start ~2.5us in,
  * all input tiles are SBUF-resident (8.4MB of 24MB), so DMA runs free
    of WAR hazards from t=0,
  * the r-gate of sub-chunk j+1 is computed one iteration early so its
    sigmoid+r*h (ScalarE+VectorE) never gate the candidate matmuls,
  * fp16 gate math on DVE (2x mode, SBUF-only), activations read PSUM
    f32 and write fp16, per-sub-chunk output DMAs keep the tail short.
"""

import os
import sys

for _p in ("/opt/trn_rl_repo", "/root/.axon_site/_ro/trn_rl_repo"):
    if os.path.isdir(_p) and _p not in sys.path:
        sys.path.append(_p)

import numpy as np

B = 65536
D = 256
N_CORES = 8
S = B // N_CORES  # batch rows per core
CH = 512  # batch columns per PSUM bank / compute sub-chunk

# Input-tile load plan: (col_start, width). The first is narrow (and split
# per block) so the pipeline head fills fast; the rest are wide packed
# loads for DMA efficiency.
PLAN = [(0, 512), (512, 512)] + [(1024 + 1024 * i, 1024) for i in range(7)]
# block order inside the packed input tensor
_BLOCKS = ("x0", "x1", "h0", "h1")
# matrix order inside the packed weight tensor
_WORDER = ("Wr", "Ur", "Wz", "Uz", "Wh", "Uh")
_BORDER = ("br", "bz", "bh")


def _sub_to_load(j):
    """Map 512-wide sub-chunk j to (load_index, local col offset)."""
    c0 = j * CH
    for li, (start, width) in enumerate(PLAN):
        if start <= c0 < start + width:
            return li, c0 - start
    raise ValueError(j)


def build_nc(s=S, mm_dtype_name=None):
    """Build + compile the per-core Bass program for a shard of s rows."""
    import concourse.bass as bass
    import concourse.mybir as mybir
    import concourse.tile as tile
    from concourse import bacc

    f32 = mybir.dt.float32
    if mm_dtype_name is None:
        mm_dtype_name = os.environ.get("GRU_MM_DTYPE", "float16")
    f16 = getattr(mybir.dt, mm_dtype_name)
    AF = mybir.ActivationFunctionType

    nc = bacc.Bacc("TRN2", target_bir_lowering=False)
    xh = nc.dram_tensor("xh", [128, 4, s], f16, kind="ExternalInput")
    wcat = nc.dram_tensor("wcat", [D, 6 * D], f16, kind="ExternalInput")
    bcat = nc.dram_tensor("bcat", [128, 6], f32, kind="ExternalInput")
    outT = nc.dram_tensor("outT", [D, s], f16, kind="ExternalOutput")

    nsub = s // CH

    with tile.TileContext(nc) as tc:
        with (
            tc.tile_pool(name="const", bufs=1) as cpool,
            tc.tile_pool(name="work", bufs=2) as wpool,
            tc.tile_pool(name="outb", bufs=4) as opool,
            tc.tile_pool(name="psum", bufs=1, space=bass.MemorySpace.PSUM) as ppool,
        ):
            inp = {}  # (block, load_idx) -> AP [128, width]

            # The framework preamble blocks all engines until ~7.2us, then
            # each engine can issue DMA triggers at ~650ns apiece.  Spread
            # the latency-critical head loads across engines so four
            # transfers launch in parallel the moment the barrier lifts.
            wA, wB = {}, {}
            wA[0] = cpool.tile([128, 2 * D], f16, tag="wA0", name="wA0")
            nc.sync.dma_start(wA[0][:], wcat[0:128, 0 : 2 * D])
            wA[1] = cpool.tile([128, 2 * D], f16, tag="wA1", name="wA1")
            nc.scalar.dma_start(wA[1][:], wcat[128:256, 0 : 2 * D])

            w0, w0w = PLAN[0]
            head_eng = {"x0": nc.gpsimd, "x1": nc.scalar,
                        "h0": nc.sync, "h1": nc.gpsimd}
            for bi, blk in enumerate(_BLOCKS):
                t = cpool.tile([128, w0w], f16, tag=f"i{blk}_0", name=f"i{blk}_0")
                head_eng[blk].dma_start(t[:], xh[:, bi, w0 : w0 + w0w])
                inp[(blk, 0)] = t
            wB[0] = cpool.tile([128, 4 * D], f16, tag="wB0", name="wB0")
            nc.gpsimd.dma_start(wB[0][:], wcat[0:128, 2 * D : 6 * D])
            wB[1] = cpool.tile([128, 4 * D], f16, tag="wB1", name="wB1")
            nc.scalar.dma_start(wB[1][:], wcat[128:256, 2 * D : 6 * D])
            b_sb = cpool.tile([128, 6], f32, tag="bcat")
            nc.sync.dma_start(b_sb[:], bcat[:, :])

            for li in range(1, len(PLAN)):
                start, width = PLAN[li]
                t = cpool.tile([128, 4, width], f16, tag=f"ixh_{li}",
                               name=f"ixh_{li}")
                eng = nc.gpsimd if li % 2 else nc.sync
                eng.dma_start(t[:], xh[:, :, start : start + width])
                for bi, blk in enumerate(_BLOCKS):
                    inp[(blk, li)] = t[:, bi, :]

            def wap(i, k, g):
                """Weight AP [128,128] for matrix index i (order _WORDER),
                contraction half k, output-feature half g."""
                if i < 2:
                    return wA[k][:, i * D + g * 128 : i * D + (g + 1) * 128]
                return wB[k][:, (i - 2) * D + g * 128 : (i - 2) * D + (g + 1) * 128]

            def operands(j):
                li, off = _sub_to_load(j)
                sl = slice(off, off + CH)
                xs = [inp[(f"x{k}", li)][:, sl] for k in range(2)]
                hs = [inp[(f"h{k}", li)][:, sl] for k in range(2)]
                return xs, hs

            def gate_psum(tag, wi, ui, xs, rhs_u, g):
                p = ppool.tile([128, CH], f32, tag=tag, name=tag)
                nc.tensor.matmul(p[:], wap(wi, 0, g), xs[0], start=True, stop=False)
                nc.tensor.matmul(p[:], wap(wi, 1, g), xs[1], start=False, stop=False)
                nc.tensor.matmul(p[:], wap(ui, 0, g), rhs_u[0], start=False, stop=False)
                nc.tensor.matmul(p[:], wap(ui, 1, g), rhs_u[1], start=False, stop=True)
                return p

            def r_gate(j):
                """reset gate -> r*h tiles for sub-chunk j."""
                xs, hs = operands(j)
                rh = []
                for g in range(2):
                    pr = gate_psum(f"pr{g}", 0, 1, xs, hs, g)
                    rt = wpool.tile([128, CH], f16, tag=f"r{g}", name=f"r{g}")
                    nc.scalar.activation(rt[:], pr[:], AF.Sigmoid,
                                         bias=b_sb[:, g : g + 1])
                    t = wpool.tile([128, CH], f16, tag=f"rh{g}", name=f"rh{g}")
                    nc.vector.tensor_mul(t[:], rt[:], hs[g])
                    rh.append(t)
                return rh

            # software pipeline: r-gate one sub-chunk ahead of z/candidate
            rh_cur = r_gate(0)
            for j in range(nsub):
                xs, hs = operands(j)
                rh_next = r_gate(j + 1) if j + 1 < nsub else None

                zt = []
                for g in range(2):
                    pz = gate_psum(f"pz{g}", 2, 3, xs, hs, g)
                    t = wpool.tile([128, CH], f16, tag=f"z{g}", name=f"z{g}")
                    nc.scalar.activation(t[:], pz[:], AF.Sigmoid,
                                         bias=b_sb[:, 2 + g : 3 + g])
                    zt.append(t)

                for g in range(2):
                    ph = gate_psum(f"ph{g}", 4, 5, xs, rh_cur, g)
                    hh = wpool.tile([128, CH], f16, tag=f"hh{g}", name=f"hh{g}")
                    nc.scalar.activation(hh[:], ph[:], AF.Tanh,
                                         bias=b_sb[:, 4 + g : 5 + g])
                    d = wpool.tile([128, CH], f16, tag=f"d{g}", name=f"d{g}")
                    nc.vector.tensor_sub(d[:], hs[g], hh[:])
                    m = wpool.tile([128, CH], f16, tag=f"m{g}", name=f"m{g}")
                    nc.vector.tensor_mul(m[:], zt[g][:], d[:])
                    o = opool.tile([128, CH], f16, tag=f"o{g}", name=f"o{g}")
                    nc.vector.tensor_add(o[:], hh[:], m[:])
                    nc.sync.dma_start(
                        outT[g * 128 : (g + 1) * 128, j * CH : (j + 1) * CH], o[:]
                    )
                rh_cur = rh_next

    nc.compile()
    return nc


_NC_CACHE = {}


def _get_nc():
    key = (S, os.environ.get("GRU_MM_DTYPE", "float16"))
    if key not in _NC_CACHE:
        _NC_CACHE[key] = build_nc(S, key[1])
    return _NC_CACHE[key]


def _make_in_maps(inputs):
    f32 = np.float32
    dt16 = {"float16": np.float16}.get(
        os.environ.get("GRU_MM_DTYPE", "float16")
    )
    if dt16 is None:
        import ml_dtypes

        dt16 = ml_dtypes.bfloat16
    x = np.asarray(inputs["x"], f32)
    h = np.asarray(inputs["h_t_1"], f32)
    wcat = np.ascontiguousarray(
        np.concatenate(
            [np.asarray(inputs[n], f32) for n in ("Wr", "Ur", "Wz", "Uz", "Wh", "Uh")],
            axis=1,
        ).astype(dt16)
    )
    bcat = np.ascontiguousarray(
        np.concatenate(
            [np.asarray(inputs[n], f32).reshape(2, 128).T for n in ("br", "bz", "bh")],
            axis=1,
        )
    )
    consts = {"wcat": wcat, "bcat": bcat}
    in_maps = []
    for c in range(N_CORES):
        sl = slice(c * S, (c + 1) * S)
        xT = x[sl].T.astype(dt16)  # [256, S]
        hT = h[sl].T.astype(dt16)
        xh = np.empty((128, 4, S), dt16)
        xh[:, 0] = xT[0:128]
        xh[:, 1] = xT[128:256]
        xh[:, 2] = hT[0:128]
        xh[:, 3] = hT[128:256]
        m = {"xh": np.ascontiguousarray(xh)}
        m.update(consts)
        in_maps.append(m)
    return in_maps


def run(inputs, trace=False):
    """Run on hardware; returns (h_t ndarray, BassKernelResults)."""
    from concourse.bass_utils import run_bass_kernel_spmd

    nc = _get_nc()
    in_maps = _make_in_maps(inputs)
    res = run_bass_kernel_spmd(nc, in_maps, list(range(N_CORES)), trace=trace)
    out = np.empty((B, D), np.float32)
    for c in range(N_CORES):
        out[c * S : (c + 1) * S] = res.results[c]["outT"].T.astype(np.float32)
    return out, res


def kernel(**inputs):
    out, _ = run(inputs, trace=False)
    return (out, out)


# revision 11
# speedup vs baseline: 1.0137x; 1.0137x over previous
"""GRU cell on 8 Trainium2 NeuronCores.

Reference computation (B=65536, D=256):
    z = sigmoid(x@Wz + h@Uz + bz)
    r = sigmoid(x@Wr + h@Ur + br)
    h_hat = tanh(x@Wh + (r*h)@Uh + bh)
    h_t = z*h + (1-z)*h_hat  ; returns (h_t, h_t)

Strategy: data-parallel over the batch dim (8 shards of 8192 rows), all
fp16 on chip (rel_l2 ~1.1e-3 vs the f32 reference; gate is 2e-2):
  * host packs each shard as [128 partitions, 4 blocks, 8192] fp16 where
    the blocks are (x k0, x k1, h k0, h k1) - the contraction dim of all
    six GEMMs is the SBUF partition dim and one DMA fetches all four
    operand tiles of a column range (DMA triggers are ~645ns each on
    SyncE, so trigger count is latency that delays the pipeline head),
  * weights packed into one [256, 1536] fp16 matrix, r-gate slice first,
    DMA-ordered so the first matmul can start ~2.5us in,
  * all input tiles are SBUF-resident (8.4MB of 24MB), so DMA runs free
    of WAR hazards from t=0,
  * the r-gate of sub-chunk j+1 is computed one iteration early so its
    sigmoid+r*h (ScalarE+VectorE) never gate the candidate matmuls,
  * fp16 gate math on DVE (2x mode, SBUF-only), activations read PSUM
    f32 and write fp16, per-sub-chunk output DMAs keep the tail short.
"""

import os
import sys

for _p in ("/opt/trn_rl_repo", "/root/.axon_site/_ro/trn_rl_repo"):
    if os.path.isdir(_p) and _p not in sys.path:
        sys.path.append(_p)

import numpy as np

B = 65536
D = 256
N_CORES = 8
S = B // N_CORES  # batch rows per core
CH = 512  # batch columns per PSUM bank / compute sub-chunk

# Input-tile load plan: (col_start, width). The first is narrow (and split
# per block) so the pipeline head fills fast; the rest are wide packed
# loads for DMA efficiency.
PLAN = [(0, 512), (512, 512), (1024, 512), (1536, 512)] + [
    (2048 + 1024 * i, 1024) for i in range(6)
]
# block order inside the packed input tensor
_BLOCKS = ("x0", "x1", "h0", "h1")
# matrix order inside the packed weight tensor
_WORDER = ("Wr", "Ur", "Wz", "Uz", "Wh", "Uh")
_BORDER = ("br", "bz", "bh")


def _sub_to_load(j):
    """Map 512-wide sub-chunk j to (load_index, local col offset)."""
    c0 = j * CH
    for li, (start, width) in enumerate(PLAN):
        if start <= c0 < start + width:
            return li, c0 - start
    raise ValueError(j)


def build_nc(s=S, mm_dtype_name=None):
    """Build + compile the per-core Bass program for a shard of s rows."""
    import concourse.bass as bass
    import concourse.mybir as mybir
    import concourse.tile as tile
    from concourse import bacc

    f32 = mybir.dt.float32
    if mm_dtype_name is None:
        mm_dtype_name = os.environ.get("GRU_MM_DTYPE", "float16")
    f16 = getattr(mybir.dt, mm_dtype_name)
    AF = mybir.ActivationFunctionType

    nc = bacc.Bacc("TRN2", target_bir_lowering=False)
    xh = nc.dram_tensor("xh", [128, 4, s], f16, kind="ExternalInput")
    wcat = nc.dram_tensor("wcat", [D, 6 * D], f16, kind="ExternalInput")
    bcat = nc.dram_tensor("bcat", [128, 6], f32, kind="ExternalInput")
    outT = nc.dram_tensor("outT", [D, s], f16, kind="ExternalOutput")

    nsub = s // CH

    with tile.TileContext(nc) as tc:
        with (
            tc.tile_pool(name="const", bufs=1) as cpool,
            tc.tile_pool(name="work", bufs=2) as wpool,
            tc.tile_pool(name="outb", bufs=4) as opool,
            tc.tile_pool(name="psum", bufs=1, space=bass.MemorySpace.PSUM) as ppool,
        ):
            inp = {}  # (block, load_idx) -> AP [128, width]

            # PE warm-up: the HAM clock gate needs ~3.4us of sustained PE
            # activity to lift the engine from 1.2 to 2.4 GHz.  The PE is
            # idle during the head DMAs anyway, so burn that window on
            # dummy matmuls over a memset tile (results go to a scratch
            # PSUM bank that is read once and discarded).
            zt0 = cpool.tile([128, CH], f16, tag="warm", name="warm")
            nc.gpsimd.memset(zt0[:], 0)
            pw = ppool.tile([128, CH], f32, tag="pwarm", name="pwarm")
            for _ in range(7):
                nc.tensor.matmul(pw[:], zt0[:, 0:128], zt0[:], start=True, stop=True)
            wsink = cpool.tile([128, CH], f32, tag="wsink", name="wsink")
            nc.vector.tensor_copy(wsink[:], pw[:])

            # The framework preamble blocks all engines until ~7.2us, then
            # each engine issues DMA triggers at ~650ns apiece and a single
            # DMA instruction only sustains ~40-70GB/s.  So: split the head
            # tiles into half-tiles dual-issued from SyncE and ScalarE (the
            # two cheap trigger engines), in exactly the order the first
            # matmuls consume them.
            wA, wB = {}, {}
            for k in range(2):
                wA[k] = cpool.tile([128, 2 * D], f16, tag=f"wA{k}", name=f"wA{k}")
                wB[k] = cpool.tile([128, 4 * D], f16, tag=f"wB{k}", name=f"wB{k}")
            w0, w0w = PLAN[0]
            hw2 = w0w // 2
            for bi, blk in enumerate(_BLOCKS):  # x0, x1, h0, h1
                t = cpool.tile([128, w0w], f16, tag=f"i{blk}_0", name=f"i{blk}_0")
                nc.sync.dma_start(t[:, 0:hw2], xh[:, bi, w0 : w0 + hw2])
                nc.scalar.dma_start(t[:, hw2:w0w], xh[:, bi, w0 + hw2 : w0 + w0w])
                inp[(blk, 0)] = t

            # sync queue: first input loads + packed stream + biases
            t = cpool.tile([128, 4, PLAN[1][1]], f16, tag="ixh_1", name="ixh_1")
            nc.sync.dma_start(t[:], xh[:, :, PLAN[1][0] : PLAN[1][0] + PLAN[1][1]])
            for bi, blk in enumerate(_BLOCKS):
                inp[(blk, 1)] = t[:, bi, :]
            nc.sync.dma_start(wB[0][:, 0:512], wcat[0:128, 512:1024])
            nc.sync.dma_start(wB[0][:, 512:1024], wcat[0:128, 1024:1536])
            b_sb = cpool.tile([128, 6], f32, tag="bcat")
            nc.sync.dma_start(b_sb[:], bcat[:, :])

            # scalar queue: r-gate weights in consumption order, then the
            # k1 halves of the z/h weights
            nc.scalar.dma_start(wA[0][:, 0:256], wcat[0:128, 0:256])
            nc.scalar.dma_start(wA[1][:, 0:256], wcat[128:256, 0:256])
            nc.scalar.dma_start(wA[0][:, 256:512], wcat[0:128, 256:512])
            nc.scalar.dma_start(wA[1][:, 256:512], wcat[128:256, 256:512])
            nc.scalar.dma_start(wB[1][:, 0:512], wcat[128:256, 512:1024])
            nc.scalar.dma_start(wB[1][:, 512:1024], wcat[128:256, 1024:1536])

            for li in range(2, len(PLAN)):
                start, width = PLAN[li]
                t = cpool.tile([128, 4, width], f16, tag=f"ixh_{li}",
                               name=f"ixh_{li}")
                nc.sync.dma_start(t[:], xh[:, :, start : start + width])
                for bi, blk in enumerate(_BLOCKS):
                    inp[(blk, li)] = t[:, bi, :]

            def wap(i, k, g):
                """Weight AP [128,128] for matrix index i (order _WORDER),
                contraction half k, output-feature half g."""
                if i < 2:
                    return wA[k][:, i * D + g * 128 : i * D + (g + 1) * 128]
                return wB[k][:, (i - 2) * D + g * 128 : (i - 2) * D + (g + 1) * 128]

            def operands(j):
                li, off = _sub_to_load(j)
                sl = slice(off, off + CH)
                xs = [inp[(f"x{k}", li)][:, sl] for k in range(2)]
                hs = [inp[(f"h{k}", li)][:, sl] for k in range(2)]
                return xs, hs

            def gate_psum(tag, wi, ui, xs, rhs_u, g):
                p = ppool.tile([128, CH], f32, tag=tag, name=tag)
                nc.tensor.matmul(p[:], wap(wi, 0, g), xs[0], start=True, stop=False)
                nc.tensor.matmul(p[:], wap(wi, 1, g), xs[1], start=False, stop=False)
                nc.tensor.matmul(p[:], wap(ui, 0, g), rhs_u[0], start=False, stop=False)
                nc.tensor.matmul(p[:], wap(ui, 1, g), rhs_u[1], start=False, stop=True)
                return p

            def r_gate(j):
                """reset gate -> r*h tiles for sub-chunk j."""
                xs, hs = operands(j)
                rh = []
                for g in range(2):
                    pr = gate_psum(f"pr{g}", 0, 1, xs, hs, g)
                    rt = wpool.tile([128, CH], f16, tag=f"r{g}", name=f"r{g}")
                    nc.scalar.activation(rt[:], pr[:], AF.Sigmoid,
                                         bias=b_sb[:, g : g + 1])
                    t = wpool.tile([128, CH], f16, tag=f"rh{g}", name=f"rh{g}")
                    nc.vector.tensor_mul(t[:], rt[:], hs[g])
                    rh.append(t)
                return rh

            # software pipeline: r-gate one sub-chunk ahead of z/candidate
            rh_cur = r_gate(0)
            for j in range(nsub):
                xs, hs = operands(j)
                rh_next = r_gate(j + 1) if j + 1 < nsub else None

                zt = []
                for g in range(2):
                    pz = gate_psum(f"pz{g}", 2, 3, xs, hs, g)
                    t = wpool.tile([128, CH], f16, tag=f"z{g}", name=f"z{g}")
                    nc.scalar.activation(t[:], pz[:], AF.Sigmoid,
                                         bias=b_sb[:, 2 + g : 3 + g])
                    zt.append(t)

                for g in range(2):
                    ph = gate_psum(f"ph{g}", 4, 5, xs, rh_cur, g)
                    hh = wpool.tile([128, CH], f16, tag=f"hh{g}", name=f"hh{g}")
                    nc.scalar.activation(hh[:], ph[:], AF.Tanh,
                                         bias=b_sb[:, 4 + g : 5 + g])
                    d = wpool.tile([128, CH], f16, tag=f"d{g}", name=f"d{g}")
                    nc.vector.tensor_sub(d[:], hs[g], hh[:])
                    m = wpool.tile([128, CH], f16, tag=f"m{g}", name=f"m{g}")
                    nc.vector.tensor_mul(m[:], zt[g][:], d[:])
                    o = opool.tile([128, CH], f16, tag=f"o{g}", name=f"o{g}")
                    nc.vector.tensor_add(o[:], hh[:], m[:])
                    orow = outT[g * 128 : (g + 1) * 128, :]
                    if j == nsub - 1:
                        # split the tail store across both trigger engines
                        # so the last transfer is half as long
                        mid = j * CH + CH // 2
                        nc.sync.dma_start(orow[:, j * CH : mid], o[:, 0 : CH // 2])
                        nc.scalar.dma_start(orow[:, mid : (j + 1) * CH],
                                            o[:, CH // 2 : CH])
                    else:
                        nc.sync.dma_start(
                            orow[:, j * CH : (j + 1) * CH], o[:]
                        )
                rh_cur = rh_next

    nc.compile()
    return nc


_NC_CACHE = {}


def _get_nc():
    key = (S, os.environ.get("GRU_MM_DTYPE", "float16"))
    if key not in _NC_CACHE:
        _NC_CACHE[key] = build_nc(S, key[1])
    return _NC_CACHE[key]


def _make_in_maps(inputs):
    f32 = np.float32
    dt16 = {"float16": np.float16}.get(
        os.environ.get("GRU_MM_DTYPE", "float16")
    )
    if dt16 is None:
        import ml_dtypes

        dt16 = ml_dtypes.bfloat16
    x = np.asarray(inputs["x"], f32)
    h = np.asarray(inputs["h_t_1"], f32)
    wcat = np.ascontiguousarray(
        np.concatenate(
            [np.asarray(inputs[n], f32) for n in ("Wr", "Ur", "Wz", "Uz", "Wh", "Uh")],
            axis=1,
        ).astype(dt16)
    )
    bcat = np.ascontiguousarray(
        np.concatenate(
            [np.asarray(inputs[n], f32).reshape(2, 128).T for n in ("br", "bz", "bh")],
            axis=1,
        )
    )
    consts = {"wcat": wcat, "bcat": bcat}
    in_maps = []
    for c in range(N_CORES):
        sl = slice(c * S, (c + 1) * S)
        xT = x[sl].T.astype(dt16)  # [256, S]
        hT = h[sl].T.astype(dt16)
        xh = np.empty((128, 4, S), dt16)
        xh[:, 0] = xT[0:128]
        xh[:, 1] = xT[128:256]
        xh[:, 2] = hT[0:128]
        xh[:, 3] = hT[128:256]
        m = {"xh": np.ascontiguousarray(xh)}
        m.update(consts)
        in_maps.append(m)
    return in_maps


def run(inputs, trace=False):
    """Run on hardware; returns (h_t ndarray, BassKernelResults)."""
    from concourse.bass_utils import run_bass_kernel_spmd

    nc = _get_nc()
    in_maps = _make_in_maps(inputs)
    res = run_bass_kernel_spmd(nc, in_maps, list(range(N_CORES)), trace=trace)
    out = np.empty((B, D), np.float32)
    for c in range(N_CORES):
        out[c * S : (c + 1) * S] = res.results[c]["outT"].T.astype(np.float32)
    return out, res


def kernel(**inputs):
    out, _ = run(inputs, trace=False)
    return (out, out)
